# revision 26
# baseline (speedup 1.0000x reference)
"""Trainium2 Bass kernel for a single-head attention block (B=4, S=2048, D=1024).

reference:
    x = gelu(tokens); q,k,v = x@W{q,k,v} + b; scores = q@k^T/sqrt(D)
    out = softmax(scores)@v @ Wo + bo + tokens

Sharding: 8 cores = 4 batches x 2 query-halves. Core c=2b+h handles batch b and
query rows [h*1024, (h+1)*1024). Each core computes q/k/v projections for its
own rows only; K^T and V halves are exchanged pairwise via two AllGathers. The
fp32 residual path dominates the output magnitude, so the whole matmul pipeline
runs in fp8-e4m3 with DoubleRow perf mode (K=256 per matmul).

KEY LAYOUT TRICK (v4): softmax is permutation-invariant over the key axis as
long as k and v use the SAME order, so each core keeps its OWN key/value rows
in tiles [0, SQ) of kT/v and the PEER's rows in [SQ, 2*SQ). The projection
evictions write straight into the own half (no copy), and the peer half is
pulled from the AllGather output with a dma_gather whose int16 row indices are
HOST-PROVIDED per-core data (peer slot = 1-h) — the program stays SPMD-uniform
while the own-half scores run with no dependency on the collective at all.

Scales: weights are pre-scaled x32 on the host (sigma~1 in fp8), so stored
q,k,v are 32x true scale. scores_psum = 32768*scores_true -> exp uses
scale=2^-15, bias=-5ln2, giving expT = exp(scores)/32 in fp8. Softmax
denominators via a ones-stationary matmul; rS_row = 1/Sigma exp is broadcast
across partitions via a DRAM round-trip (hidden behind the other chunk's
scores). The mixed psum is normalized on the psum->fp8 DVE convert (v is
centered host-side so the fp8 mixUT quantizes the small AC part). The out-proj
psum is 32*(mixed@Wo)*32, folded by 1/1024 on the fused
(psum*c + residual) DVE op; bo and the centering correction are pre-added into
the residual on the host.

Schedule: PSUM evictions alternate ACT/DVE; PE order is
  warmup | kTo -> AG1 | vo -> AG2 | qT | sc0-own sc0-peer S0 | sc1-own
  sc1-peer S1 | mix0 out0 | mix1 out1
so the AllGather wire+gather latency hides behind qT+own-half scores, and each
chunk's softmax reciprocal round-trip hides behind the other chunk's work.
"""

import math

import numpy as np
import ml_dtypes

B, S, D = 4, 2048, 1024
NCORES = 8
SQ = S // 2          # query rows per core
P = 128
DT = 8               # d / 128
KP = DT // 2         # K-pair count for DoubleRow (K=256 each)
ST = S // P          # 16 seq tiles
SQT = SQ // P        # 8
N512 = 512
WARMUP_MMS = 34
WSCALE = 32.0        # host-side weight/bias scale
EXP_BIAS = -5.0 * math.log(2.0)   # expT = exp(scores)/32
EXP_SCALE = 1.0 / 32768.0         # scores_psum = 32768 * scores_true
OUT_DESCALE = 1.0 / 1024.0
GELU_MEAN = 0.3989422804014327    # E[gelu(z)], z ~ N(0,1)

_COMPILED = {}


def _build_program():
    from contextlib import ExitStack

    import concourse.bass as bass
    import concourse.tile as tile
    from concourse import bacc, mybir

    f32 = mybir.dt.float32
    bf16 = mybir.dt.bfloat16
    f8 = mybir.dt.float8e4
    i16 = mybir.dt.int16
    AF = mybir.ActivationFunctionType
    ALU = mybir.AluOpType
    DR = mybir.MatmulPerfMode.DoubleRow

    nc = bacc.Bacc("TRN2", target_bir_lowering=False, debug=False,
                   num_devices=NCORES)

    tokTq = nc.dram_tensor("tokTq", [D, SQ], bf16, kind="ExternalInput")
    resid = nc.dram_tensor("resid", [SQ, D], bf16, kind="ExternalInput")
    wq = nc.dram_tensor("wq", [D, D], f8, kind="ExternalInput")
    wk = nc.dram_tensor("wk", [D, D], f8, kind="ExternalInput")
    wv = nc.dram_tensor("wv", [D, D], f8, kind="ExternalInput")
    wo = nc.dram_tensor("wo", [D, D], f8, kind="ExternalInput")
    bq_d = nc.dram_tensor("bq", [D], f32, kind="ExternalInput")   # x32
    bk_d = nc.dram_tensor("bk", [D], f32, kind="ExternalInput")   # x32
    bv_d = nc.dram_tensor("bv", [D], f32, kind="ExternalInput")   # x32
    gidxk_d = nc.dram_tensor("gidxk", [P, S // 64], i16, kind="ExternalInput")
    out_d = nc.dram_tensor("out", [SQ, D], f32, kind="ExternalOutput")

    ts = bass.ts
    groups = [[2 * i, 2 * i + 1] for i in range(NCORES // 2)]

    with tile.TileContext(nc) as tc, ExitStack() as ctx:
        pers = ctx.enter_context(tc.tile_pool(name="pers", bufs=1))
        kT = pers.tile([P, DT, SQ], f8, tag="kT")     # own keys
        kTp = pers.tile([P, DT, SQ], f8, tag="kTp")   # peer keys
        qT = pers.tile([P, DT, SQ], f8, tag="qT")
        v = pers.tile([P, SQT, D], f8, tag="v")       # own values
        vp = pers.tile([P, SQT, D], f8, tag="vp")     # peer values
        ones = pers.tile([P, 2, 16], f8, tag="ones")
        bqk = pers.tile([P, 2, DT], f32, tag="bqk")  # [:,0,:]=32bq [:,1,:]=32bk
        ebias = pers.tile([P, 1], f32, tag="ebias")
        wscr = pers.tile([P, N512], bf16, tag="wscr")
        wsink = pers.tile([P, P], f32, tag="wsink")
        wo_sb = pers.tile([P, DT, D], f8, tag="wo")
        gidxk = pers.tile([P, S // 64], i16, tag="gidxk")

        dram = ctx.enter_context(tc.tile_pool(name="dram", bufs=1, space="DRAM"))
        kb_in_a = dram.tile([D // 2, SQ], f8, tag="kb_in_a")
        kb_in_b = dram.tile([D // 2, SQ], f8, tag="kb_in_b")
        kb_out_a = dram.tile([2, D // 2, SQ], f8, tag="kb_out_a")
        kb_out_b = dram.tile([2, D // 2, SQ], f8, tag="kb_out_b")
        vb_in_a = dram.tile([SQ // 2, D], f8, tag="vb_in_a")
        vb_in_b = dram.tile([SQ // 2, D], f8, tag="vb_in_b")
        vb_out_a = dram.tile([2, SQ // 2, D], f8, tag="vb_out_a")
        vb_out_b = dram.tile([2, SQ // 2, D], f8, tag="vb_out_b")

        psum = ctx.enter_context(tc.tile_pool(name="psum", bufs=7, space="PSUM"))
        psum_s = ctx.enter_context(tc.tile_pool(name="psum_s", bufs=1, space="PSUM"))

        # --- PE warm-up: dense trivial matmuls so HAM hits K=8/8 and PE is
        # busy while the gelu+DMA head runs.
        nc.vector.memset(wscr, 0.0)
        wps = psum.tile([P, N512], f32, tag="mm")
        for i in range(WARMUP_MMS):
            nc.tensor.matmul(wps, wscr[:, :P], wscr, start=(i == 0),
                             stop=(i == WARMUP_MMS - 1))
        nc.vector.tensor_copy(wsink, wps[:, :P])

        nc.vector.memset(ones, 1.0)
        nc.vector.memset(ebias, EXP_BIAS)
        nc.scalar.dma_start(bqk[:, 0, :], bq_d.ap().rearrange("(t p) -> p t", p=P))
        nc.scalar.dma_start(bqk[:, 1, :], bk_d.ap().rearrange("(t p) -> p t", p=P))

        # ---------------- phase 1: gelu + projections + kT/v exchange -------
        with ExitStack() as ph1:
            p1 = ph1.enter_context(tc.tile_pool(name="p1", bufs=1))
            xTq = p1.tile([P, DT, SQ], f8, tag="xTq")
            wk_sb = p1.tile([P, DT, D], f8, tag="wk")
            wq_sb = p1.tile([P, DT, D], f8, tag="wq")
            wv_sb = p1.tile([P, DT, D], f8, tag="wv")
            bv_sb = p1.tile([P, D], f32, tag="bv")
            stag = ph1.enter_context(tc.tile_pool(name="stag", bufs=4))

            # Head is HBM-bound: load ONLY what the gelu needs now (tokens +
            # Wk); Wv/Wq/Wo triggers are interleaved into the staging loops
            # below so their 3MB doesn't steal HBM bandwidth from the tokens.
            nc.gpsimd.dma_start(wk_sb,
                                wk.ap().rearrange("(t p) e -> p t e", p=P))
            nc.gpsimd.dma_start(
                bv_sb, bass.AP(tensor=bv_d, offset=0, ap=[[0, P], [1, D]]))
            nc.gpsimd.dma_start(gidxk, gidxk_d.ap())
            # tokens in 4 pair-tiles; gelu per pair so each ACT op unlocks a
            # full DoubleRow K-pair for the projection matmuls
            for g in range(DT // 2):
                stq = stag.tile([P, 2, SQ], bf16, tag="tok", name=f"stq{g}")
                if g == 0:
                    # first pair split in two so the gelu chain starts a DMA
                    # half-transfer earlier
                    for hh in range(2):
                        nc.sync.dma_start(
                            stq[:, hh, :],
                            tokTq.ap()[(2 * g + hh) * P:(2 * g + hh + 1) * P, :])
                        nc.scalar.activation(xTq[:, 2 * g + hh, :],
                                             stq[:, hh, :], AF.Gelu)
                else:
                    nc.sync.dma_start(
                        stq, tokTq.ap()[2 * g * P:(2 * g + 2) * P, :].rearrange(
                            "(t p) s -> p t s", p=P))
                    nc.scalar.activation(xTq[:, 2 * g:2 * g + 2, :], stq,
                                         AF.Gelu)

            # kTo: lhsT = Wk-slice, rhs = xTq -> write own half of kT
            # directly; stream each te row-block to DRAM as its converts
            # land. The exchange is split into two half-AllGathers so the
            # first fires as soon as te 0-3 are staged.
            kb_in_av = kb_in_a[:].rearrange("(t p) s -> p t s", p=P)
            kb_in_bv = kb_in_b[:].rearrange("(t p) s -> p t s", p=P)
            for te in range(DT):
                for c in range(SQ // N512):
                    ps = psum.tile([P, N512], f32, tag="mm")
                    for u in range(KP):
                        nc.tensor.matmul(ps, wk_sb[:, 2 * u:2 * u + 2, ts(te, P)],
                                         xTq[:, 2 * u:2 * u + 2, ts(c, N512)],
                                         start=(u == 0), stop=(u == KP - 1),
                                         perf_mode=DR)
                    if c == 0:
                        nc.scalar.activation(kT[:, te, ts(c, N512)], ps,
                                             AF.Identity, bias=bqk[:, 1, te:te + 1])
                    else:
                        nc.vector.tensor_scalar_add(kT[:, te, ts(c, N512)], ps,
                                                    bqk[:, 1, te:te + 1])
                kb_v = kb_in_av if te < 4 else kb_in_bv
                nc.sync.dma_start(kb_v[:, te % 4, :], kT[:, te, :])
                if te == 0:
                    nc.sync.dma_start(
                        wv_sb, wv.ap().rearrange("(t p) e -> p t e", p=P))
                elif te == 3:
                    nc.sync.dma_start(
                        wq_sb, wq.ap().rearrange("(t p) e -> p t e", p=P))
                elif te == DT - 1:
                    pass
                if te == 3:
                    nc.gpsimd.collective_compute(
                        "AllGather", mybir.AluOpType.bypass,
                        replica_groups=groups,
                        ins=[kb_in_a[:].opt()], outs=[kb_out_a[:].opt()])
            nc.gpsimd.collective_compute(
                "AllGather", mybir.AluOpType.bypass, replica_groups=groups,
                ins=[kb_in_b[:].opt()], outs=[kb_out_b[:].opt()])

            # vo : lhsT = xTq-slice, rhs = Wv -> own half of v; exchange
            # split in two half-AllGathers like the keys
            vb_in_av = vb_in_a[:].rearrange("(t p) d -> p t d", p=P)
            vb_in_bv = vb_in_b[:].rearrange("(t p) d -> p t d", p=P)
            for tsq in range(SQT):
                for dc in range(D // N512):
                    ps = psum.tile([P, N512], f32, tag="mm")
                    for u in range(KP):
                        nc.tensor.matmul(ps, xTq[:, 2 * u:2 * u + 2, ts(tsq, P)],
                                         wv_sb[:, 2 * u:2 * u + 2, ts(dc, N512)],
                                         start=(u == 0), stop=(u == KP - 1),
                                         perf_mode=DR)
                    nc.vector.tensor_add(v[:, tsq, ts(dc, N512)], ps,
                                         bv_sb[:, ts(dc, N512)])
                vb_v = vb_in_av if tsq < 4 else vb_in_bv
                nc.sync.dma_start(vb_v[:, tsq % 4, :], v[:, tsq, :])
                if tsq == 1:
                    nc.sync.dma_start(
                        wo_sb, wo.ap().rearrange("(t p) e -> p t e", p=P))
                if tsq == 3:
                    nc.gpsimd.collective_compute(
                        "AllGather", mybir.AluOpType.bypass,
                        replica_groups=groups,
                        ins=[vb_in_a[:].opt()], outs=[vb_out_a[:].opt()])
            nc.gpsimd.collective_compute(
                "AllGather", mybir.AluOpType.bypass, replica_groups=groups,
                ins=[vb_in_b[:].opt()], outs=[vb_out_b[:].opt()])

            # peer halves: gather the peer's rows of the AllGather outputs
            # straight into the peer tiles — idx data is per-core
            nc.gpsimd.dma_gather(kTp[:, 0:4, :],
                                 kb_out_a[:].rearrange("r d s -> (r d) s"),
                                 gidxk[:, :], S // 4, S // 4, SQ)
            nc.gpsimd.dma_gather(kTp[:, 4:8, :],
                                 kb_out_b[:].rearrange("r d s -> (r d) s"),
                                 gidxk[:, :], S // 4, S // 4, SQ)
            nc.gpsimd.dma_gather(vp[:, 0:4, :],
                                 vb_out_a[:].rearrange("r s d -> (r s) d"),
                                 gidxk[:, :], S // 4, S // 4, D)
            nc.gpsimd.dma_gather(vp[:, 4:8, :],
                                 vb_out_b[:].rearrange("r s d -> (r s) d"),
                                 gidxk[:, :], S // 4, S // 4, D)

            # qT : lhsT = Wq-slice, rhs = xTq
            for te in range(DT):
                for c in range(SQ // N512):
                    ps = psum.tile([P, N512], f32, tag="mm")
                    for u in range(KP):
                        nc.tensor.matmul(ps, wq_sb[:, 2 * u:2 * u + 2, ts(te, P)],
                                         xTq[:, 2 * u:2 * u + 2, ts(c, N512)],
                                         start=(u == 0), stop=(u == KP - 1),
                                         perf_mode=DR)
                    if c == 0:
                        nc.scalar.activation(qT[:, te, ts(c, N512)], ps,
                                             AF.Identity, bias=bqk[:, 0, te:te + 1])
                    else:
                        nc.vector.tensor_scalar_add(qT[:, te, ts(c, N512)], ps,
                                                    bqk[:, 0, te:te + 1])

        # ---------------- phase 2: attention + out-proj ----------------
        with ExitStack() as ph2:
            epool = ph2.enter_context(tc.tile_pool(name="ep", bufs=2))
            work = ph2.enter_context(tc.tile_pool(name="wk2", bufs=2))
            opool = ph2.enter_context(tc.tile_pool(name="op2", bufs=2))
            rspool = ph2.enter_context(tc.tile_pool(name="rs2", bufs=2))
            rpool = ph2.enter_context(tc.tile_pool(name="rp", bufs=8))
            dpool = ph2.enter_context(
                tc.tile_pool(name="dram2", bufs=2, space="DRAM"))

            # scores in own-half / peer-half blocks; each chunk's softmax
            # denominator + reciprocal round-trip hides behind later blocks
            expTs, rSbs = [], []
            for c in range(SQ // N512):          # sq chunks of 512
                expT = epool.tile([P, ST, N512], f8, tag="expT",
                                  name=f"expT{c}")
                expTs.append(expT)

            def sc_block(c, tk_lo, tk_hi):
                expT = expTs[c]
                for tk in range(tk_lo, tk_hi):
                    ksrc = kT if tk < SQT else kTp
                    ps = psum.tile([P, N512], f32, tag="mm")
                    for u in range(KP):
                        nc.tensor.matmul(ps,
                                         ksrc[:, 2 * u:2 * u + 2,
                                              ts(tk % SQT, P)],
                                         qT[:, 2 * u:2 * u + 2, ts(c, N512)],
                                         start=(u == 0), stop=(u == KP - 1),
                                         perf_mode=DR)
                    nc.scalar.activation(expT[:, tk, :], ps, AF.Exp,
                                         scale=EXP_SCALE, bias=ebias)

            def s_block(c):
                expT = expTs[c]
                psS = psum_s.tile([1, N512], f32, tag="S")
                for tk in range(ST // 2):
                    nc.tensor.matmul(psS, ones[:, :, :1],
                                     expT[:, 2 * tk:2 * tk + 2, :],
                                     start=(tk == 0), stop=(tk == ST // 2 - 1),
                                     perf_mode=DR)
                rS_row = rspool.tile([1, N512], f32, tag="rS_row",
                                     name=f"rS{c}")
                nc.vector.reciprocal(rS_row, psS)   # = 32 / Sigma exp
                # broadcast 1/S across partitions via DRAM (stride-0 DMA)
                rs_dram = dpool.tile([N512], f32, tag="rs_dram")
                nc.sync.dma_start(
                    rs_dram[:].rearrange("(o s) -> o s", o=1), rS_row)
                rSb = rspool.tile([P, N512], f32, tag="rSb", name=f"rSb{c}")
                nc.scalar.dma_start(rSb, rs_dram[:].partition_broadcast(P))
                rSbs.append(rSb)

            sc_block(0, 0, SQT)        # own keys: no collective dependency
            sc_block(1, 0, SQT)        # more own-key work to hide the wire
            sc_block(0, SQT, ST)       # peer keys: needs AG1 + gathers
            s_block(0)
            sc_block(1, SQT, ST)
            s_block(1)

            # residual prefetch AFTER the exchange window so its HBM reads
            # don't fight the AllGather wire + gathers (bf16: half traffic)
            res_sbs = []
            for sl8 in range(SQT):
                res_sb = rpool.tile([P, D], bf16, tag="res", name=f"res{sl8}")
                nc.sync.dma_start(res_sb, resid.ap()[sl8 * P:(sl8 + 1) * P, :])
                res_sbs.append(res_sb)

            for c in range(SQ // N512):
                expT, rSb = expTs[c], rSbs[c]
                # mixedUT[d, sq] = (v^T-stationary @ expT) / S  (normalized on
                # the psum->fp8 convert; unnormalized would overflow e4m3).
                # For chunk 0 the peer values may still be in flight, so six
                # dsl groups run their own-half accumulations first (banks
                # held open) to cover the tail of the v-exchange with work.
                mixUT = work.tile([P, DT, N512], f8, tag="mixUT",
                                  name=f"mixUT{c}")
                nheld = 7 if c == 0 else 0
                held = []
                for dsl in range(nheld):
                    ps = psum.tile([P, N512], f32, tag="mm")
                    for tk in range(SQT // 2):
                        nc.tensor.matmul(ps, v[:, 2 * tk:2 * tk + 2, ts(dsl, P)],
                                         expT[:, 2 * tk:2 * tk + 2, :],
                                         start=(tk == 0), stop=False,
                                         perf_mode=DR)
                    held.append(ps)
                for dsl in range(nheld):
                    ps = held[dsl]
                    for tk in range(SQT // 2, ST // 2):
                        nc.tensor.matmul(ps,
                                         vp[:, (2 * tk) % SQT:
                                             (2 * tk) % SQT + 2, ts(dsl, P)],
                                         expT[:, 2 * tk:2 * tk + 2, :],
                                         start=False, stop=(tk == ST // 2 - 1),
                                         perf_mode=DR)
                    nc.vector.tensor_mul(mixUT[:, dsl, :], ps, rSb)
                for dsl in range(nheld, DT):
                    ps = psum.tile([P, N512], f32, tag="mm")
                    for tk in range(ST // 2):
                        vsrc = v if tk < SQT // 2 else vp
                        nc.tensor.matmul(ps,
                                         vsrc[:, (2 * tk) % SQT:
                                              (2 * tk) % SQT + 2, ts(dsl, P)],
                                         expT[:, 2 * tk:2 * tk + 2, :],
                                         start=(tk == 0), stop=(tk == ST // 2 - 1),
                                         perf_mode=DR)
                    nc.vector.tensor_mul(mixUT[:, dsl, :], ps, rSb)

                for sl in range(4):
                    row = (c * 4 + sl) * P
                    res_sb = res_sbs[c * 4 + sl]
                    out_sb = opool.tile([P, D], f32, tag="osb")
                    osc = opool.tile([P, N512], f32, tag="osc")
                    for ec in range(D // N512):
                        ps = psum.tile([P, N512], f32, tag="mm")
                        for u in range(KP):
                            nc.tensor.matmul(
                                ps, mixUT[:, 2 * u:2 * u + 2, ts(sl, P)],
                                wo_sb[:, 2 * u:2 * u + 2, ts(ec, N512)],
                                start=(u == 0), stop=(u == KP - 1),
                                perf_mode=DR)
                        # out = psum / 1024 + (residual + bo); alternate the
                        # evict between DVE (fused) and ACT+GpSimd
                        if ec == 0:
                            nc.vector.scalar_tensor_tensor(
                                out_sb[:, ts(ec, N512)], ps, OUT_DESCALE,
                                res_sb[:, ts(ec, N512)], ALU.mult, ALU.add)
                        else:
                            nc.scalar.activation(osc, ps, AF.Identity,
                                                 scale=OUT_DESCALE)
                            nc.gpsimd.tensor_add(out_sb[:, ts(ec, N512)], osc,
                                                 res_sb[:, ts(ec, N512)])
                    nc.sync.dma_start(out_d.ap()[row:row + P, :], out_sb)

    nc.compile()
    return nc


def _get_program():
    if "nc" not in _COMPILED:
        _COMPILED["nc"] = _build_program()
    return _COMPILED["nc"]


def make_in_maps(tokens, Wq, bq, Wk, bk, Wv, bv, Wo, bo):
    tokens = np.asarray(tokens, dtype=np.float32)
    bf = ml_dtypes.bfloat16
    f8 = ml_dtypes.float8_e4m3
    wq_b = np.ascontiguousarray((np.asarray(Wq, np.float32) * WSCALE).astype(f8))
    wk_b = np.ascontiguousarray((np.asarray(Wk, np.float32) * WSCALE).astype(f8))
    wv_b = np.ascontiguousarray((np.asarray(Wv, np.float32) * WSCALE).astype(f8))
    wo_b = np.ascontiguousarray((np.asarray(Wo, np.float32) * WSCALE).astype(f8))
    bq = np.asarray(bq, np.float32) * WSCALE
    bk = np.asarray(bk, np.float32) * WSCALE
    # center v by c ~ E_k[v] so the fp8 mixUT quantizes the small AC part;
    # softmax weights sum to 1, so out = (mixed-c)@Wo + (c@Wo + bo) + resid.
    wv32 = np.asarray(Wv, np.float32)
    cvec = GELU_MEAN * wv32.sum(axis=0) + np.asarray(bv, np.float32)
    bv = (np.asarray(bv, np.float32) - cvec) * WSCALE
    bo_eff = (np.asarray(bo, np.float32)
              + cvec @ np.asarray(Wo, np.float32)).astype(np.float32)

    pp, mm = np.meshgrid(np.arange(P), np.arange(S // 64), indexing="ij")
    base_k = (mm * 16 + (pp % 16)).astype(np.int16)     # j = m*16 + lane

    in_maps = []
    for c in range(NCORES):
        b, h = divmod(c, 2)
        q_rows = tokens[b, h * SQ:(h + 1) * SQ]
        in_maps.append({
            "tokTq": np.ascontiguousarray(q_rows.T.astype(bf)),  # [D, SQ]
            "resid": np.ascontiguousarray((q_rows + bo_eff).astype(bf)),
            "wq": wq_b, "wk": wk_b, "wv": wv_b, "wo": wo_b,
            "bq": bq, "bk": bk, "bv": bv,
            "gidxk": np.ascontiguousarray(base_k + np.int16((1 - h) * (SQ // 2))),
        })
    return in_maps


def gather_out(results):
    out = np.empty((B, S, D), np.float32)
    for c in range(NCORES):
        b, h = divmod(c, 2)
        out[b, h * SQ:(h + 1) * SQ] = results[c]["out"]
    return out


def kernel(tokens, Wq, bq, Wk, bk, Wv, bv, Wo, bo):
    from concourse.bass_utils import run_bass_kernel_spmd

    in_maps = make_in_maps(tokens, Wq, bq, Wk, bk, Wv, bv, Wo, bo)
    nc = _get_program()
    res = run_bass_kernel_spmd(nc, in_maps, core_ids=list(range(NCORES)),
                               trace=False)
    return gather_out(res.results)


# revision 27
# speedup vs baseline: 1.0152x; 1.0152x over previous
"""Trainium2 Bass kernel for a single-head attention block (B=4, S=2048, D=1024).

reference:
    x = gelu(tokens); q,k,v = x@W{q,k,v} + b; scores = q@k^T/sqrt(D)
    out = softmax(scores)@v @ Wo + bo + tokens

Sharding: 8 cores = 4 batches x 2 query-halves. Core c=2b+h handles batch b and
query rows [h*1024, (h+1)*1024). Each core computes q/k/v projections for its
own rows only; K^T and V halves are exchanged pairwise via two AllGathers. The
fp32 residual path dominates the output magnitude, so the whole matmul pipeline
runs in fp8-e4m3 with DoubleRow perf mode (K=256 per matmul).

KEY LAYOUT TRICK (v4): softmax is permutation-invariant over the key axis as
long as k and v use the SAME order, so each core keeps its OWN key/value rows
in tiles [0, SQ) of kT/v and the PEER's rows in [SQ, 2*SQ). The projection
evictions write straight into the own half (no copy), and the peer half is
pulled from the AllGather output with a dma_gather whose int16 row indices are
HOST-PROVIDED per-core data (peer slot = 1-h) — the program stays SPMD-uniform
while the own-half scores run with no dependency on the collective at all.

Scales: weights are pre-scaled x32 on the host (sigma~1 in fp8), so stored
q,k,v are 32x true scale. scores_psum = 32768*scores_true -> exp uses
scale=2^-15, bias=-5ln2, giving expT = exp(scores)/32 in fp8. Softmax
denominators via a ones-stationary matmul; rS_row = 1/Sigma exp is broadcast
across partitions via a DRAM round-trip (hidden behind the other chunk's
scores). The mixed psum is normalized on the psum->fp8 DVE convert (v is
centered host-side so the fp8 mixUT quantizes the small AC part). The out-proj
psum is 32*(mixed@Wo)*32, folded by 1/1024 on the fused
(psum*c + residual) DVE op; bo and the centering correction are pre-added into
the residual on the host.

Schedule: PSUM evictions alternate ACT/DVE; PE order is
  warmup | kTo -> AG1 | vo -> AG2 | qT | sc0-own sc0-peer S0 | sc1-own
  sc1-peer S1 | mix0 out0 | mix1 out1
so the AllGather wire+gather latency hides behind qT+own-half scores, and each
chunk's softmax reciprocal round-trip hides behind the other chunk's work.
"""

import math

import numpy as np
import ml_dtypes

B, S, D = 4, 2048, 1024
NCORES = 8
SQ = S // 2          # query rows per core
P = 128
DT = 8               # d / 128
KP = DT // 2         # K-pair count for DoubleRow (K=256 each)
ST = S // P          # 16 seq tiles
SQT = SQ // P        # 8
N512 = 512
WARMUP_MMS = 34
WSCALE = 32.0        # host-side weight/bias scale
EXP_BIAS = -5.0 * math.log(2.0)   # expT = exp(scores)/32
EXP_SCALE = 1.0 / 32768.0         # scores_psum = 32768 * scores_true
OUT_DESCALE = 1.0 / 1024.0
GELU_MEAN = 0.3989422804014327    # E[gelu(z)], z ~ N(0,1)

_COMPILED = {}


def _build_program():
    from contextlib import ExitStack

    import concourse.bass as bass
    import concourse.tile as tile
    from concourse import bacc, mybir

    f32 = mybir.dt.float32
    bf16 = mybir.dt.bfloat16
    f8 = mybir.dt.float8e4
    i16 = mybir.dt.int16
    AF = mybir.ActivationFunctionType
    ALU = mybir.AluOpType
    DR = mybir.MatmulPerfMode.DoubleRow

    nc = bacc.Bacc("TRN2", target_bir_lowering=False, debug=False,
                   num_devices=NCORES)

    tokTq = nc.dram_tensor("tokTq", [D, SQ], bf16, kind="ExternalInput")
    resid = nc.dram_tensor("resid", [SQ, D], bf16, kind="ExternalInput")
    wq = nc.dram_tensor("wq", [D, D], f8, kind="ExternalInput")
    wk = nc.dram_tensor("wk", [D, D], f8, kind="ExternalInput")
    wv = nc.dram_tensor("wv", [D, D], f8, kind="ExternalInput")
    wo = nc.dram_tensor("wo", [D, D], f8, kind="ExternalInput")
    bq_d = nc.dram_tensor("bq", [D], f32, kind="ExternalInput")   # x32
    bk_d = nc.dram_tensor("bk", [D], f32, kind="ExternalInput")   # x32
    bv_d = nc.dram_tensor("bv", [D], f32, kind="ExternalInput")   # x32
    gidxk_d = nc.dram_tensor("gidxk", [P, S // 64], i16, kind="ExternalInput")
    out_d = nc.dram_tensor("out", [SQ, D], f32, kind="ExternalOutput")

    ts = bass.ts
    groups = [[2 * i, 2 * i + 1] for i in range(NCORES // 2)]

    with tile.TileContext(nc) as tc, ExitStack() as ctx:
        pers = ctx.enter_context(tc.tile_pool(name="pers", bufs=1))
        kT = pers.tile([P, DT, SQ], f8, tag="kT")     # own keys
        kTp = pers.tile([P, DT, SQ], f8, tag="kTp")   # peer keys
        qT = pers.tile([P, DT, SQ], f8, tag="qT")
        v = pers.tile([P, SQT, D], f8, tag="v")       # own values
        vp = pers.tile([P, SQT, D], f8, tag="vp")     # peer values
        ones = pers.tile([P, 2, 16], f8, tag="ones")
        bqk = pers.tile([P, 2, DT], f32, tag="bqk")  # [:,0,:]=32bq [:,1,:]=32bk
        ebias = pers.tile([P, 1], f32, tag="ebias")
        wscr = pers.tile([P, N512], bf16, tag="wscr")
        wsink = pers.tile([P, P], f32, tag="wsink")
        wo_sb = pers.tile([P, DT, D], f8, tag="wo")
        gidxk = pers.tile([P, S // 64], i16, tag="gidxk")

        dram = ctx.enter_context(tc.tile_pool(name="dram", bufs=1, space="DRAM"))
        kb_in_a = dram.tile([D // 2, SQ], f8, tag="kb_in_a")
        kb_in_b = dram.tile([D // 2, SQ], f8, tag="kb_in_b")
        kb_out_a = dram.tile([2, D // 2, SQ], f8, tag="kb_out_a")
        kb_out_b = dram.tile([2, D // 2, SQ], f8, tag="kb_out_b")
        vb_in_a = dram.tile([SQ // 2, D], f8, tag="vb_in_a")
        vb_in_b = dram.tile([SQ // 2, D], f8, tag="vb_in_b")
        vb_out_a = dram.tile([2, SQ // 2, D], f8, tag="vb_out_a")
        vb_out_b = dram.tile([2, SQ // 2, D], f8, tag="vb_out_b")

        psum = ctx.enter_context(tc.tile_pool(name="psum", bufs=6, space="PSUM"))
        psum_s = ctx.enter_context(tc.tile_pool(name="psum_s", bufs=2, space="PSUM"))

        # --- PE warm-up: dense trivial matmuls so HAM hits K=8/8 and PE is
        # busy while the gelu+DMA head runs.
        nc.vector.memset(wscr, 0.0)
        wps = psum.tile([P, N512], f32, tag="mm")
        for i in range(WARMUP_MMS):
            nc.tensor.matmul(wps, wscr[:, :P], wscr, start=(i == 0),
                             stop=(i == WARMUP_MMS - 1))
        nc.vector.tensor_copy(wsink, wps[:, :P])

        nc.vector.memset(ones, 1.0)
        nc.vector.memset(ebias, EXP_BIAS)
        nc.scalar.dma_start(bqk[:, 0, :], bq_d.ap().rearrange("(t p) -> p t", p=P))
        nc.scalar.dma_start(bqk[:, 1, :], bk_d.ap().rearrange("(t p) -> p t", p=P))

        # ---------------- phase 1: gelu + projections + kT/v exchange -------
        with ExitStack() as ph1:
            p1 = ph1.enter_context(tc.tile_pool(name="p1", bufs=1))
            xTq = p1.tile([P, DT, SQ], f8, tag="xTq")
            wk_sb = p1.tile([P, DT, D], f8, tag="wk")
            wq_sb = p1.tile([P, DT, D], f8, tag="wq")
            wv_sb = p1.tile([P, DT, D], f8, tag="wv")
            bv_sb = p1.tile([P, D], f32, tag="bv")
            stag = ph1.enter_context(tc.tile_pool(name="stag", bufs=4))

            # Head is HBM-bound: load ONLY what the gelu needs now (tokens +
            # Wk); Wv/Wq/Wo triggers are interleaved into the staging loops
            # below so their 3MB doesn't steal HBM bandwidth from the tokens.
            nc.gpsimd.dma_start(wk_sb,
                                wk.ap().rearrange("(t p) e -> p t e", p=P))
            nc.gpsimd.dma_start(
                bv_sb, bass.AP(tensor=bv_d, offset=0, ap=[[0, P], [1, D]]))
            nc.gpsimd.dma_start(gidxk, gidxk_d.ap())
            # tokens in 4 pair-tiles; gelu per pair so each ACT op unlocks a
            # full DoubleRow K-pair for the projection matmuls
            for g in range(DT // 2):
                stq = stag.tile([P, 2, SQ], bf16, tag="tok", name=f"stq{g}")
                nc.sync.dma_start(
                    stq, tokTq.ap()[2 * g * P:(2 * g + 2) * P, :].rearrange(
                        "(t p) s -> p t s", p=P))
                nc.scalar.activation(xTq[:, 2 * g:2 * g + 2, :], stq, AF.Gelu)

            # kTo: lhsT = Wk-slice, rhs = xTq -> write own half of kT
            # directly; stream each te row-block to DRAM as its converts
            # land. The exchange is split into two half-AllGathers so the
            # first fires as soon as te 0-3 are staged.
            kb_in_av = kb_in_a[:].rearrange("(t p) s -> p t s", p=P)
            kb_in_bv = kb_in_b[:].rearrange("(t p) s -> p t s", p=P)
            for te in range(DT):
                for c in range(SQ // N512):
                    ps = psum.tile([P, N512], f32, tag="mm")
                    for u in range(KP):
                        nc.tensor.matmul(ps, wk_sb[:, 2 * u:2 * u + 2, ts(te, P)],
                                         xTq[:, 2 * u:2 * u + 2, ts(c, N512)],
                                         start=(u == 0), stop=(u == KP - 1),
                                         perf_mode=DR)
                    if c == 0:
                        nc.scalar.activation(kT[:, te, ts(c, N512)], ps,
                                             AF.Identity, bias=bqk[:, 1, te:te + 1])
                    else:
                        nc.vector.tensor_scalar_add(kT[:, te, ts(c, N512)], ps,
                                                    bqk[:, 1, te:te + 1])
                kb_v = kb_in_av if te < 4 else kb_in_bv
                nc.sync.dma_start(kb_v[:, te % 4, :], kT[:, te, :])
                if te == 0:
                    nc.sync.dma_start(
                        wv_sb, wv.ap().rearrange("(t p) e -> p t e", p=P))
                elif te == 3:
                    nc.sync.dma_start(
                        wq_sb, wq.ap().rearrange("(t p) e -> p t e", p=P))
                elif te == DT - 1:
                    pass
                if te == 3:
                    nc.gpsimd.collective_compute(
                        "AllGather", mybir.AluOpType.bypass,
                        replica_groups=groups,
                        ins=[kb_in_a[:].opt()], outs=[kb_out_a[:].opt()])
            nc.gpsimd.collective_compute(
                "AllGather", mybir.AluOpType.bypass, replica_groups=groups,
                ins=[kb_in_b[:].opt()], outs=[kb_out_b[:].opt()])

            # vo : lhsT = xTq-slice, rhs = Wv -> own half of v; exchange
            # split in two half-AllGathers like the keys
            vb_in_av = vb_in_a[:].rearrange("(t p) d -> p t d", p=P)
            vb_in_bv = vb_in_b[:].rearrange("(t p) d -> p t d", p=P)
            for tsq in range(SQT):
                for dc in range(D // N512):
                    ps = psum.tile([P, N512], f32, tag="mm")
                    for u in range(KP):
                        nc.tensor.matmul(ps, xTq[:, 2 * u:2 * u + 2, ts(tsq, P)],
                                         wv_sb[:, 2 * u:2 * u + 2, ts(dc, N512)],
                                         start=(u == 0), stop=(u == KP - 1),
                                         perf_mode=DR)
                    nc.vector.tensor_add(v[:, tsq, ts(dc, N512)], ps,
                                         bv_sb[:, ts(dc, N512)])
                vb_v = vb_in_av if tsq < 4 else vb_in_bv
                nc.sync.dma_start(vb_v[:, tsq % 4, :], v[:, tsq, :])
                if tsq == 1:
                    nc.sync.dma_start(
                        wo_sb, wo.ap().rearrange("(t p) e -> p t e", p=P))
                if tsq == 3:
                    nc.gpsimd.collective_compute(
                        "AllGather", mybir.AluOpType.bypass,
                        replica_groups=groups,
                        ins=[vb_in_a[:].opt()], outs=[vb_out_a[:].opt()])
            nc.gpsimd.collective_compute(
                "AllGather", mybir.AluOpType.bypass, replica_groups=groups,
                ins=[vb_in_b[:].opt()], outs=[vb_out_b[:].opt()])

            # peer halves: gather the peer's rows of the AllGather outputs
            # straight into the peer tiles — idx data is per-core
            nc.gpsimd.dma_gather(kTp[:, 0:4, :],
                                 kb_out_a[:].rearrange("r d s -> (r d) s"),
                                 gidxk[:, :], S // 4, S // 4, SQ)
            nc.gpsimd.dma_gather(kTp[:, 4:8, :],
                                 kb_out_b[:].rearrange("r d s -> (r d) s"),
                                 gidxk[:, :], S // 4, S // 4, SQ)
            nc.gpsimd.dma_gather(vp[:, 0:4, :],
                                 vb_out_a[:].rearrange("r s d -> (r s) d"),
                                 gidxk[:, :], S // 4, S // 4, D)
            nc.gpsimd.dma_gather(vp[:, 4:8, :],
                                 vb_out_b[:].rearrange("r s d -> (r s) d"),
                                 gidxk[:, :], S // 4, S // 4, D)

            # qT : lhsT = Wq-slice, rhs = xTq
            for te in range(DT):
                for c in range(SQ // N512):
                    ps = psum.tile([P, N512], f32, tag="mm")
                    for u in range(KP):
                        nc.tensor.matmul(ps, wq_sb[:, 2 * u:2 * u + 2, ts(te, P)],
                                         xTq[:, 2 * u:2 * u + 2, ts(c, N512)],
                                         start=(u == 0), stop=(u == KP - 1),
                                         perf_mode=DR)
                    if c == 0:
                        nc.scalar.activation(qT[:, te, ts(c, N512)], ps,
                                             AF.Identity, bias=bqk[:, 0, te:te + 1])
                    else:
                        nc.vector.tensor_scalar_add(qT[:, te, ts(c, N512)], ps,
                                                    bqk[:, 0, te:te + 1])

        # ---------------- phase 2: attention + out-proj ----------------
        with ExitStack() as ph2:
            epool = ph2.enter_context(tc.tile_pool(name="ep", bufs=2))
            work = ph2.enter_context(tc.tile_pool(name="wk2", bufs=2))
            opool = ph2.enter_context(tc.tile_pool(name="op2", bufs=2))
            rspool = ph2.enter_context(tc.tile_pool(name="rs2", bufs=2))
            rpool = ph2.enter_context(tc.tile_pool(name="rp", bufs=8))
            dpool = ph2.enter_context(
                tc.tile_pool(name="dram2", bufs=2, space="DRAM"))

            # scores in own-half / peer-half blocks; each chunk's softmax
            # denominator + reciprocal round-trip hides behind later blocks
            expTs, rSbs = [], []
            for c in range(SQ // N512):          # sq chunks of 512
                expT = epool.tile([P, ST, N512], f8, tag="expT",
                                  name=f"expT{c}")
                expTs.append(expT)

            def sc_block(c, tk_lo, tk_hi):
                expT = expTs[c]
                for tk in range(tk_lo, tk_hi):
                    ksrc = kT if tk < SQT else kTp
                    ps = psum.tile([P, N512], f32, tag="mm")
                    for u in range(KP):
                        nc.tensor.matmul(ps,
                                         ksrc[:, 2 * u:2 * u + 2,
                                              ts(tk % SQT, P)],
                                         qT[:, 2 * u:2 * u + 2, ts(c, N512)],
                                         start=(u == 0), stop=(u == KP - 1),
                                         perf_mode=DR)
                    nc.scalar.activation(expT[:, tk, :], ps, AF.Exp,
                                         scale=EXP_SCALE, bias=ebias)

            def s_block(c):
                expT = expTs[c]
                psS = psum_s.tile([1, N512], f32, tag="S")
                for tk in range(ST // 2):
                    nc.tensor.matmul(psS, ones[:, :, :1],
                                     expT[:, 2 * tk:2 * tk + 2, :],
                                     start=(tk == 0), stop=(tk == ST // 2 - 1),
                                     perf_mode=DR)
                rS_row = rspool.tile([1, N512], f32, tag="rS_row",
                                     name=f"rS{c}")
                nc.vector.reciprocal(rS_row, psS)   # = 32 / Sigma exp
                # broadcast 1/S across partitions via DRAM (stride-0 DMA)
                rs_dram = dpool.tile([N512], f32, tag="rs_dram")
                nc.sync.dma_start(
                    rs_dram[:].rearrange("(o s) -> o s", o=1), rS_row)
                rSb = rspool.tile([P, N512], f32, tag="rSb", name=f"rSb{c}")
                nc.scalar.dma_start(rSb, rs_dram[:].partition_broadcast(P))
                rSbs.append(rSb)

            sc_block(0, 0, SQT)        # own keys: no collective dependency
            sc_block(1, 0, SQT)        # more own-key work to hide the wire
            sc_block(0, SQT, ST)       # peer keys: needs AG1 + gathers
            s_block(0)
            sc_block(1, SQT, ST)
            s_block(1)

            # residual prefetch AFTER the exchange window so its HBM reads
            # don't fight the AllGather wire + gathers (bf16: half traffic)
            res_sbs = []
            for sl8 in range(SQT):
                res_sb = rpool.tile([P, D], bf16, tag="res", name=f"res{sl8}")
                nc.sync.dma_start(res_sb, resid.ap()[sl8 * P:(sl8 + 1) * P, :])
                res_sbs.append(res_sb)

            for c in range(SQ // N512):
                expT, rSb = expTs[c], rSbs[c]
                # mixedUT[d, sq] = (v^T-stationary @ expT) / S  (normalized on
                # the psum->fp8 convert; unnormalized would overflow e4m3).
                # For chunk 0 the peer values may still be in flight, so six
                # dsl groups run their own-half accumulations first (banks
                # held open) to cover the tail of the v-exchange with work.
                mixUT = work.tile([P, DT, N512], f8, tag="mixUT",
                                  name=f"mixUT{c}")
                nheld = 6 if c == 0 else 0
                held = []
                for dsl in range(nheld):
                    ps = psum.tile([P, N512], f32, tag="mm")
                    for tk in range(SQT // 2):
                        nc.tensor.matmul(ps, v[:, 2 * tk:2 * tk + 2, ts(dsl, P)],
                                         expT[:, 2 * tk:2 * tk + 2, :],
                                         start=(tk == 0), stop=False,
                                         perf_mode=DR)
                    held.append(ps)
                for dsl in range(nheld):
                    ps = held[dsl]
                    for tk in range(SQT // 2, ST // 2):
                        nc.tensor.matmul(ps,
                                         vp[:, (2 * tk) % SQT:
                                             (2 * tk) % SQT + 2, ts(dsl, P)],
                                         expT[:, 2 * tk:2 * tk + 2, :],
                                         start=False, stop=(tk == ST // 2 - 1),
                                         perf_mode=DR)
                    nc.vector.tensor_mul(mixUT[:, dsl, :], ps, rSb)
                for dsl in range(nheld, DT):
                    ps = psum.tile([P, N512], f32, tag="mm")
                    for tk in range(ST // 2):
                        vsrc = v if tk < SQT // 2 else vp
                        nc.tensor.matmul(ps,
                                         vsrc[:, (2 * tk) % SQT:
                                              (2 * tk) % SQT + 2, ts(dsl, P)],
                                         expT[:, 2 * tk:2 * tk + 2, :],
                                         start=(tk == 0), stop=(tk == ST // 2 - 1),
                                         perf_mode=DR)
                    nc.vector.tensor_mul(mixUT[:, dsl, :], ps, rSb)

                for sl in range(4):
                    row = (c * 4 + sl) * P
                    res_sb = res_sbs[c * 4 + sl]
                    out_sb = opool.tile([P, D], f32, tag="osb")
                    osc = opool.tile([P, N512], f32, tag="osc")
                    for ec in range(D // N512):
                        ps = psum.tile([P, N512], f32, tag="mm")
                        for u in range(KP):
                            nc.tensor.matmul(
                                ps, mixUT[:, 2 * u:2 * u + 2, ts(sl, P)],
                                wo_sb[:, 2 * u:2 * u + 2, ts(ec, N512)],
                                start=(u == 0), stop=(u == KP - 1),
                                perf_mode=DR)
                        # out = psum / 1024 + (residual + bo); alternate the
                        # evict between DVE (fused) and ACT+GpSimd
                        if ec == 0:
                            nc.vector.scalar_tensor_tensor(
                                out_sb[:, ts(ec, N512)], ps, OUT_DESCALE,
                                res_sb[:, ts(ec, N512)], ALU.mult, ALU.add)
                        else:
                            nc.scalar.activation(osc, ps, AF.Identity,
                                                 scale=OUT_DESCALE)
                            nc.gpsimd.tensor_add(out_sb[:, ts(ec, N512)], osc,
                                                 res_sb[:, ts(ec, N512)])
                    nc.sync.dma_start(out_d.ap()[row:row + P, :], out_sb)

    nc.compile()
    return nc


def _get_program():
    if "nc" not in _COMPILED:
        _COMPILED["nc"] = _build_program()
    return _COMPILED["nc"]


def make_in_maps(tokens, Wq, bq, Wk, bk, Wv, bv, Wo, bo):
    tokens = np.asarray(tokens, dtype=np.float32)
    bf = ml_dtypes.bfloat16
    f8 = ml_dtypes.float8_e4m3
    wq_b = np.ascontiguousarray((np.asarray(Wq, np.float32) * WSCALE).astype(f8))
    wk_b = np.ascontiguousarray((np.asarray(Wk, np.float32) * WSCALE).astype(f8))
    wv_b = np.ascontiguousarray((np.asarray(Wv, np.float32) * WSCALE).astype(f8))
    wo_b = np.ascontiguousarray((np.asarray(Wo, np.float32) * WSCALE).astype(f8))
    bq = np.asarray(bq, np.float32) * WSCALE
    bk = np.asarray(bk, np.float32) * WSCALE
    # center v by c ~ E_k[v] so the fp8 mixUT quantizes the small AC part;
    # softmax weights sum to 1, so out = (mixed-c)@Wo + (c@Wo + bo) + resid.
    wv32 = np.asarray(Wv, np.float32)
    cvec = GELU_MEAN * wv32.sum(axis=0) + np.asarray(bv, np.float32)
    bv = (np.asarray(bv, np.float32) - cvec) * WSCALE
    bo_eff = (np.asarray(bo, np.float32)
              + cvec @ np.asarray(Wo, np.float32)).astype(np.float32)

    pp, mm = np.meshgrid(np.arange(P), np.arange(S // 64), indexing="ij")
    base_k = (mm * 16 + (pp % 16)).astype(np.int16)     # j = m*16 + lane

    in_maps = []
    for c in range(NCORES):
        b, h = divmod(c, 2)
        q_rows = tokens[b, h * SQ:(h + 1) * SQ]
        in_maps.append({
            "tokTq": np.ascontiguousarray(q_rows.T.astype(bf)),  # [D, SQ]
            "resid": np.ascontiguousarray((q_rows + bo_eff).astype(bf)),
            "wq": wq_b, "wk": wk_b, "wv": wv_b, "wo": wo_b,
            "bq": bq, "bk": bk, "bv": bv,
            "gidxk": np.ascontiguousarray(base_k + np.int16((1 - h) * (SQ // 2))),
        })
    return in_maps


def gather_out(results):
    out = np.empty((B, S, D), np.float32)
    for c in range(NCORES):
        b, h = divmod(c, 2)
        out[b, h * SQ:(h + 1) * SQ] = results[c]["out"]
    return out


def kernel(tokens, Wq, bq, Wk, bk, Wv, bv, Wo, bo):
    from concourse.bass_utils import run_bass_kernel_spmd

    in_maps = make_in_maps(tokens, Wq, bq, Wk, bk, Wv, bv, Wo, bo)
    nc = _get_program()
    res = run_bass_kernel_spmd(nc, in_maps, core_ids=list(range(NCORES)),
                               trace=False)
    return gather_out(res.results)


# revision 28
# speedup vs baseline: 1.1263x; 1.1094x over previous
"""Trainium2 Bass kernel for a single-head attention block (B=4, S=2048, D=1024).

reference:
    x = gelu(tokens); q,k,v = x@W{q,k,v} + b; scores = q@k^T/sqrt(D)
    out = softmax(scores)@v @ Wo + bo + tokens

Sharding: 8 cores = 4 batches x 2 query-halves. Core c=2b+h handles batch b and
query rows [h*1024, (h+1)*1024). Each core computes q/k/v projections for its
own rows only; K^T and V halves are exchanged pairwise via two AllGathers. The
fp32 residual path dominates the output magnitude, so the whole matmul pipeline
runs in fp8-e4m3 with DoubleRow perf mode (K=256 per matmul).

KEY LAYOUT TRICK (v4): softmax is permutation-invariant over the key axis as
long as k and v use the SAME order, so each core keeps its OWN key/value rows
in tiles [0, SQ) of kT/v and the PEER's rows in [SQ, 2*SQ). The projection
evictions write straight into the own half (no copy), and the peer half is
pulled from the AllGather output with a dma_gather whose int16 row indices are
HOST-PROVIDED per-core data (peer slot = 1-h) — the program stays SPMD-uniform
while the own-half scores run with no dependency on the collective at all.

Scales: weights are pre-scaled x32 on the host (sigma~1 in fp8), so stored
q,k,v are 32x true scale. scores_psum = 32768*scores_true -> exp uses
scale=2^-15, bias=-5ln2, giving expT = exp(scores)/32 in fp8. Softmax
denominators via a ones-stationary matmul; rS_row = 1/Sigma exp is broadcast
across partitions via a DRAM round-trip (hidden behind the other chunk's
scores). The mixed psum is normalized on the psum->fp8 DVE convert (v is
centered host-side so the fp8 mixUT quantizes the small AC part). The out-proj
psum is 32*(mixed@Wo)*32, folded by 1/1024 on the fused
(psum*c + residual) DVE op; bo and the centering correction are pre-added into
the residual on the host.

Schedule: PSUM evictions alternate ACT/DVE; PE order is
  warmup | kTo -> AG1 | vo -> AG2 | qT | sc0-own sc0-peer S0 | sc1-own
  sc1-peer S1 | mix0 out0 | mix1 out1
so the AllGather wire+gather latency hides behind qT+own-half scores, and each
chunk's softmax reciprocal round-trip hides behind the other chunk's work.
"""

import math

import numpy as np
import ml_dtypes

B, S, D = 4, 2048, 1024
NCORES = 8
SQ = S // 2          # query rows per core
P = 128
DT = 8               # d / 128
KP = DT // 2         # K-pair count for DoubleRow (K=256 each)
ST = S // P          # 16 seq tiles
SQT = SQ // P        # 8
N512 = 512
WARMUP_MMS = 34
WSCALE = 32.0        # host-side weight/bias scale
EXP_BIAS = -5.0 * math.log(2.0)   # expT = exp(scores)/32
EXP_SCALE = 1.0 / 32768.0         # scores_psum = 32768 * scores_true
OUT_DESCALE = 1.0 / 1024.0
GELU_MEAN = 0.3989422804014327    # E[gelu(z)], z ~ N(0,1)

_COMPILED = {}


def _build_program():
    from contextlib import ExitStack

    import concourse.bass as bass
    import concourse.tile as tile
    from concourse import bacc, mybir

    f32 = mybir.dt.float32
    bf16 = mybir.dt.bfloat16
    f8 = mybir.dt.float8e4
    i16 = mybir.dt.int16
    AF = mybir.ActivationFunctionType
    ALU = mybir.AluOpType
    DR = mybir.MatmulPerfMode.DoubleRow

    nc = bacc.Bacc("TRN2", target_bir_lowering=False, debug=False,
                   num_devices=NCORES)

    tokTq = nc.dram_tensor("tokTq", [D, SQ], bf16, kind="ExternalInput")
    resid = nc.dram_tensor("resid", [SQ, D], bf16, kind="ExternalInput")
    wq = nc.dram_tensor("wq", [D, D], f8, kind="ExternalInput")
    wk = nc.dram_tensor("wk", [D, D], f8, kind="ExternalInput")
    wv = nc.dram_tensor("wv", [D, D], f8, kind="ExternalInput")
    wo = nc.dram_tensor("wo", [D, D], f8, kind="ExternalInput")
    bq_d = nc.dram_tensor("bq", [D], f32, kind="ExternalInput")   # x32
    bk_d = nc.dram_tensor("bk", [D], f32, kind="ExternalInput")   # x32
    bv_d = nc.dram_tensor("bv", [D], f32, kind="ExternalInput")   # x32
    gidxk_d = nc.dram_tensor("gidxk", [P, S // 64], i16, kind="ExternalInput")
    out_d = nc.dram_tensor("out", [SQ, D], f32, kind="ExternalOutput")

    ts = bass.ts
    groups = [[2 * i, 2 * i + 1] for i in range(NCORES // 2)]

    with tile.TileContext(nc) as tc, ExitStack() as ctx:
        pers = ctx.enter_context(tc.tile_pool(name="pers", bufs=1))
        kT = pers.tile([P, DT, SQ], f8, tag="kT")     # own keys
        kTp = pers.tile([P, DT, SQ], f8, tag="kTp")   # peer keys
        qT = pers.tile([P, DT, SQ], f8, tag="qT")
        v = pers.tile([P, SQT, D], f8, tag="v")       # own values
        vp = pers.tile([P, SQT, D], f8, tag="vp")     # peer values
        ones = pers.tile([P, 2, 16], f8, tag="ones")
        bqk = pers.tile([P, 2, DT], f32, tag="bqk")  # [:,0,:]=32bq [:,1,:]=32bk
        ebias = pers.tile([P, 1], f32, tag="ebias")
        wscr = pers.tile([P, N512], bf16, tag="wscr")
        wsink = pers.tile([P, P], f32, tag="wsink")
        wo_sb = pers.tile([P, DT, D], f8, tag="wo")
        gidxk = pers.tile([P, S // 64], i16, tag="gidxk")

        dram = ctx.enter_context(tc.tile_pool(name="dram", bufs=1, space="DRAM"))
        kb_in_a = dram.tile([D // 2, SQ], f8, tag="kb_in_a")
        kb_in_b = dram.tile([D // 2, SQ], f8, tag="kb_in_b")
        kb_out_a = dram.tile([2, D // 2, SQ], f8, tag="kb_out_a")
        kb_out_b = dram.tile([2, D // 2, SQ], f8, tag="kb_out_b")
        vb_in_a = dram.tile([SQ // 2, D], f8, tag="vb_in_a")
        vb_in_b = dram.tile([SQ // 2, D], f8, tag="vb_in_b")
        vb_out_a = dram.tile([2, SQ // 2, D], f8, tag="vb_out_a")
        vb_out_b = dram.tile([2, SQ // 2, D], f8, tag="vb_out_b")

        psum = ctx.enter_context(tc.tile_pool(name="psum", bufs=6, space="PSUM"))
        psum_s = ctx.enter_context(tc.tile_pool(name="psum_s", bufs=2, space="PSUM"))

        # --- PE warm-up: dense trivial matmuls so HAM hits K=8/8 and PE is
        # busy while the gelu+DMA head runs.
        nc.vector.memset(wscr, 0.0)
        wps = psum.tile([P, N512], f32, tag="mm")
        for i in range(WARMUP_MMS):
            nc.tensor.matmul(wps, wscr[:, :P], wscr, start=(i == 0),
                             stop=(i == WARMUP_MMS - 1))
        nc.vector.tensor_copy(wsink, wps[:, :P])

        nc.vector.memset(ones, 1.0)
        nc.vector.memset(ebias, EXP_BIAS)
        nc.scalar.dma_start(bqk[:, 0, :], bq_d.ap().rearrange("(t p) -> p t", p=P))
        nc.scalar.dma_start(bqk[:, 1, :], bk_d.ap().rearrange("(t p) -> p t", p=P))

        # ---------------- phase 1: gelu + projections + kT/v exchange -------
        with ExitStack() as ph1:
            p1 = ph1.enter_context(tc.tile_pool(name="p1", bufs=1))
            xTq = p1.tile([P, DT, SQ], f8, tag="xTq")
            wk_sb = p1.tile([P, DT, D], f8, tag="wk")
            wq_sb = p1.tile([P, DT, D], f8, tag="wq")
            wv_sb = p1.tile([P, DT, D], f8, tag="wv")
            bv_sb = p1.tile([P, D], f32, tag="bv")
            stag = ph1.enter_context(tc.tile_pool(name="stag", bufs=4))

            # Head is HBM-bound: load ONLY what the gelu needs now (tokens +
            # Wk); Wv/Wq/Wo triggers are interleaved into the staging loops
            # below so their 3MB doesn't steal HBM bandwidth from the tokens.
            nc.gpsimd.dma_start(wk_sb,
                                wk.ap().rearrange("(t p) e -> p t e", p=P))
            nc.gpsimd.dma_start(
                bv_sb, bass.AP(tensor=bv_d, offset=0, ap=[[0, P], [1, D]]))
            nc.gpsimd.dma_start(gidxk, gidxk_d.ap())
            # tokens in 4 pair-tiles; gelu per pair so each ACT op unlocks a
            # full DoubleRow K-pair for the projection matmuls
            for g in range(DT // 2):
                stq = stag.tile([P, 2, SQ], bf16, tag="tok", name=f"stq{g}")
                nc.sync.dma_start(
                    stq, tokTq.ap()[2 * g * P:(2 * g + 2) * P, :].rearrange(
                        "(t p) s -> p t s", p=P))
                nc.scalar.activation(xTq[:, 2 * g:2 * g + 2, :], stq, AF.Gelu)

            # kTo: lhsT = Wk-slice, rhs = xTq -> write own half of kT
            # directly; stream each te row-block to DRAM as its converts
            # land. The exchange is split into two half-AllGathers so the
            # first fires as soon as te 0-3 are staged.
            kb_in_av = kb_in_a[:].rearrange("(t p) s -> p t s", p=P)
            kb_in_bv = kb_in_b[:].rearrange("(t p) s -> p t s", p=P)
            for te in range(DT):
                for c in range(SQ // N512):
                    ps = psum.tile([P, N512], f32, tag="mm")
                    for u in range(KP):
                        nc.tensor.matmul(ps, wk_sb[:, 2 * u:2 * u + 2, ts(te, P)],
                                         xTq[:, 2 * u:2 * u + 2, ts(c, N512)],
                                         start=(u == 0), stop=(u == KP - 1),
                                         perf_mode=DR)
                    if c == 0:
                        nc.scalar.activation(kT[:, te, ts(c, N512)], ps,
                                             AF.Identity, bias=bqk[:, 1, te:te + 1])
                    else:
                        nc.vector.tensor_scalar_add(kT[:, te, ts(c, N512)], ps,
                                                    bqk[:, 1, te:te + 1])
                kb_v = kb_in_av if te < 4 else kb_in_bv
                nc.sync.dma_start(kb_v[:, te % 4, :], kT[:, te, :])
                if te == 0:
                    nc.sync.dma_start(
                        wv_sb, wv.ap().rearrange("(t p) e -> p t e", p=P))
                elif te == 3:
                    nc.sync.dma_start(
                        wq_sb, wq.ap().rearrange("(t p) e -> p t e", p=P))
                elif te == DT - 1:
                    pass
                if te == 3:
                    nc.gpsimd.collective_compute(
                        "AllGather", mybir.AluOpType.bypass,
                        replica_groups=groups,
                        ins=[kb_in_a[:].opt()], outs=[kb_out_a[:].opt()])
            nc.gpsimd.collective_compute(
                "AllGather", mybir.AluOpType.bypass, replica_groups=groups,
                ins=[kb_in_b[:].opt()], outs=[kb_out_b[:].opt()])

            # vo : lhsT = xTq-slice, rhs = Wv -> own half of v; exchange
            # split in two half-AllGathers like the keys
            vb_in_av = vb_in_a[:].rearrange("(t p) d -> p t d", p=P)
            vb_in_bv = vb_in_b[:].rearrange("(t p) d -> p t d", p=P)
            for tsq in range(SQT):
                for dc in range(D // N512):
                    ps = psum.tile([P, N512], f32, tag="mm")
                    for u in range(KP):
                        nc.tensor.matmul(ps, xTq[:, 2 * u:2 * u + 2, ts(tsq, P)],
                                         wv_sb[:, 2 * u:2 * u + 2, ts(dc, N512)],
                                         start=(u == 0), stop=(u == KP - 1),
                                         perf_mode=DR)
                    nc.vector.tensor_add(v[:, tsq, ts(dc, N512)], ps,
                                         bv_sb[:, ts(dc, N512)])
                vb_v = vb_in_av if tsq < 4 else vb_in_bv
                nc.sync.dma_start(vb_v[:, tsq % 4, :], v[:, tsq, :])
                if tsq == 1:
                    nc.sync.dma_start(
                        wo_sb, wo.ap().rearrange("(t p) e -> p t e", p=P))
                if tsq == 3:
                    nc.gpsimd.collective_compute(
                        "AllGather", mybir.AluOpType.bypass,
                        replica_groups=groups,
                        ins=[vb_in_a[:].opt()], outs=[vb_out_a[:].opt()])
            nc.gpsimd.collective_compute(
                "AllGather", mybir.AluOpType.bypass, replica_groups=groups,
                ins=[vb_in_b[:].opt()], outs=[vb_out_b[:].opt()])

            # peer halves: gather the peer's rows of the AllGather outputs
            # straight into the peer tiles — idx data is per-core
            nc.gpsimd.dma_gather(kTp[:, 0:4, :],
                                 kb_out_a[:].rearrange("r d s -> (r d) s"),
                                 gidxk[:, :], S // 4, S // 4, SQ)
            nc.gpsimd.dma_gather(kTp[:, 4:8, :],
                                 kb_out_b[:].rearrange("r d s -> (r d) s"),
                                 gidxk[:, :], S // 4, S // 4, SQ)
            nc.gpsimd.dma_gather(vp[:, 0:4, :],
                                 vb_out_a[:].rearrange("r s d -> (r s) d"),
                                 gidxk[:, :], S // 4, S // 4, D)
            nc.gpsimd.dma_gather(vp[:, 4:8, :],
                                 vb_out_b[:].rearrange("r s d -> (r s) d"),
                                 gidxk[:, :], S // 4, S // 4, D)

            # qT : lhsT = Wq-slice, rhs = xTq
            for te in range(DT):
                for c in range(SQ // N512):
                    ps = psum.tile([P, N512], f32, tag="mm")
                    for u in range(KP):
                        nc.tensor.matmul(ps, wq_sb[:, 2 * u:2 * u + 2, ts(te, P)],
                                         xTq[:, 2 * u:2 * u + 2, ts(c, N512)],
                                         start=(u == 0), stop=(u == KP - 1),
                                         perf_mode=DR)
                    if c == 0:
                        nc.scalar.activation(qT[:, te, ts(c, N512)], ps,
                                             AF.Identity, bias=bqk[:, 0, te:te + 1])
                    else:
                        nc.vector.tensor_scalar_add(qT[:, te, ts(c, N512)], ps,
                                                    bqk[:, 0, te:te + 1])

        # ---------------- phase 2: attention + out-proj ----------------
        with ExitStack() as ph2:
            epool = ph2.enter_context(tc.tile_pool(name="ep", bufs=2))
            work = ph2.enter_context(tc.tile_pool(name="wk2", bufs=2))
            opool = ph2.enter_context(tc.tile_pool(name="op2", bufs=2))
            rspool = ph2.enter_context(tc.tile_pool(name="rs2", bufs=2))
            rpool = ph2.enter_context(tc.tile_pool(name="rp", bufs=8))
            dpool = ph2.enter_context(
                tc.tile_pool(name="dram2", bufs=2, space="DRAM"))

            # scores in own-half / peer-half blocks; each chunk's softmax
            # denominator + reciprocal round-trip hides behind later blocks
            expTs, rSbs = [], []
            for c in range(SQ // N512):          # sq chunks of 512
                expT = epool.tile([P, ST, N512], f8, tag="expT",
                                  name=f"expT{c}")
                expTs.append(expT)

            def sc_block(c, tk_lo, tk_hi):
                expT = expTs[c]
                for tk in range(tk_lo, tk_hi):
                    ksrc = kT if tk < SQT else kTp
                    ps = psum.tile([P, N512], f32, tag="mm")
                    for u in range(KP):
                        nc.tensor.matmul(ps,
                                         ksrc[:, 2 * u:2 * u + 2,
                                              ts(tk % SQT, P)],
                                         qT[:, 2 * u:2 * u + 2, ts(c, N512)],
                                         start=(u == 0), stop=(u == KP - 1),
                                         perf_mode=DR)
                    nc.scalar.activation(expT[:, tk, :], ps, AF.Exp,
                                         scale=EXP_SCALE, bias=ebias)

            def s_block(c):
                expT = expTs[c]
                psS = psum_s.tile([1, N512], f32, tag="S")
                for tk in range(ST // 2):
                    nc.tensor.matmul(psS, ones[:, :, :1],
                                     expT[:, 2 * tk:2 * tk + 2, :],
                                     start=(tk == 0), stop=(tk == ST // 2 - 1),
                                     perf_mode=DR)
                rS_row = rspool.tile([1, N512], f32, tag="rS_row",
                                     name=f"rS{c}")
                nc.vector.reciprocal(rS_row, psS)   # = 32 / Sigma exp
                # broadcast 1/S across partitions via DRAM (stride-0 DMA)
                rs_dram = dpool.tile([N512], f32, tag="rs_dram")
                nc.sync.dma_start(
                    rs_dram[:].rearrange("(o s) -> o s", o=1), rS_row)
                rSb = rspool.tile([P, N512], f32, tag="rSb", name=f"rSb{c}")
                nc.scalar.dma_start(rSb, rs_dram[:].partition_broadcast(P))
                rSbs.append(rSb)

            sc_block(0, 0, SQT)        # own keys: no collective dependency
            sc_block(1, 0, SQT)        # more own-key work to hide the wire
            sc_block(0, SQT, ST)       # peer keys: needs AG1 + gathers
            s_block(0)
            sc_block(1, SQT, ST)
            s_block(1)

            # residual prefetch AFTER the exchange window so its HBM reads
            # don't fight the AllGather wire + gathers (bf16: half traffic)
            res_sbs = []
            for sl8 in range(SQT):
                res_sb = rpool.tile([P, D], bf16, tag="res", name=f"res{sl8}")
                nc.sync.dma_start(res_sb, resid.ap()[sl8 * P:(sl8 + 1) * P, :])
                res_sbs.append(res_sb)

            for c in range(SQ // N512):
                expT, rSb = expTs[c], rSbs[c]
                # mixedUT[d, sq] = (v^T-stationary @ expT) / S  (normalized on
                # the psum->fp8 convert; unnormalized would overflow e4m3).
                # For chunk 0 the peer values may still be in flight, so six
                # dsl groups run their own-half accumulations first (banks
                # held open) to cover the tail of the v-exchange with work.
                mixUT = work.tile([P, DT, N512], f8, tag="mixUT",
                                  name=f"mixUT{c}")
                nheld = 6 if c == 0 else 0
                held = []
                for dsl in range(nheld):
                    ps = psum.tile([P, N512], f32, tag="mm")
                    for tk in range(SQT // 2):
                        nc.tensor.matmul(ps, v[:, 2 * tk:2 * tk + 2, ts(dsl, P)],
                                         expT[:, 2 * tk:2 * tk + 2, :],
                                         start=(tk == 0), stop=False,
                                         perf_mode=DR)
                    held.append(ps)
                # peer half tk-major: all held groups consume the early
                # vp tiles (first half-gather) before any needs the late ones
                for tk in range(SQT // 2, ST // 2):
                    for dsl in range(nheld):
                        nc.tensor.matmul(held[dsl],
                                         vp[:, (2 * tk) % SQT:
                                             (2 * tk) % SQT + 2, ts(dsl, P)],
                                         expT[:, 2 * tk:2 * tk + 2, :],
                                         start=False, stop=(tk == ST // 2 - 1),
                                         perf_mode=DR)
                for dsl in range(nheld):
                    nc.vector.tensor_mul(mixUT[:, dsl, :], held[dsl], rSb)
                for dsl in range(nheld, DT):
                    ps = psum.tile([P, N512], f32, tag="mm")
                    for tk in range(ST // 2):
                        vsrc = v if tk < SQT // 2 else vp
                        nc.tensor.matmul(ps,
                                         vsrc[:, (2 * tk) % SQT:
                                              (2 * tk) % SQT + 2, ts(dsl, P)],
                                         expT[:, 2 * tk:2 * tk + 2, :],
                                         start=(tk == 0), stop=(tk == ST // 2 - 1),
                                         perf_mode=DR)
                    nc.vector.tensor_mul(mixUT[:, dsl, :], ps, rSb)

                for sl in range(4):
                    row = (c * 4 + sl) * P
                    res_sb = res_sbs[c * 4 + sl]
                    out_sb = opool.tile([P, D], f32, tag="osb")
                    osc = opool.tile([P, N512], f32, tag="osc")
                    for ec in range(D // N512):
                        ps = psum.tile([P, N512], f32, tag="mm")
                        for u in range(KP):
                            nc.tensor.matmul(
                                ps, mixUT[:, 2 * u:2 * u + 2, ts(sl, P)],
                                wo_sb[:, 2 * u:2 * u + 2, ts(ec, N512)],
                                start=(u == 0), stop=(u == KP - 1),
                                perf_mode=DR)
                        # out = psum / 1024 + (residual + bo); alternate the
                        # evict between DVE (fused) and ACT+GpSimd
                        if ec == 0:
                            nc.vector.scalar_tensor_tensor(
                                out_sb[:, ts(ec, N512)], ps, OUT_DESCALE,
                                res_sb[:, ts(ec, N512)], ALU.mult, ALU.add)
                        else:
                            nc.scalar.activation(osc, ps, AF.Identity,
                                                 scale=OUT_DESCALE)
                            nc.gpsimd.tensor_add(out_sb[:, ts(ec, N512)], osc,
                                                 res_sb[:, ts(ec, N512)])
                    nc.sync.dma_start(out_d.ap()[row:row + P, :], out_sb)

    nc.compile()
    return nc


def _get_program():
    if "nc" not in _COMPILED:
        _COMPILED["nc"] = _build_program()
    return _COMPILED["nc"]


def make_in_maps(tokens, Wq, bq, Wk, bk, Wv, bv, Wo, bo):
    tokens = np.asarray(tokens, dtype=np.float32)
    bf = ml_dtypes.bfloat16
    f8 = ml_dtypes.float8_e4m3
    wq_b = np.ascontiguousarray((np.asarray(Wq, np.float32) * WSCALE).astype(f8))
    wk_b = np.ascontiguousarray((np.asarray(Wk, np.float32) * WSCALE).astype(f8))
    wv_b = np.ascontiguousarray((np.asarray(Wv, np.float32) * WSCALE).astype(f8))
    wo_b = np.ascontiguousarray((np.asarray(Wo, np.float32) * WSCALE).astype(f8))
    bq = np.asarray(bq, np.float32) * WSCALE
    bk = np.asarray(bk, np.float32) * WSCALE
    # center v by c ~ E_k[v] so the fp8 mixUT quantizes the small AC part;
    # softmax weights sum to 1, so out = (mixed-c)@Wo + (c@Wo + bo) + resid.
    wv32 = np.asarray(Wv, np.float32)
    cvec = GELU_MEAN * wv32.sum(axis=0) + np.asarray(bv, np.float32)
    bv = (np.asarray(bv, np.float32) - cvec) * WSCALE
    bo_eff = (np.asarray(bo, np.float32)
              + cvec @ np.asarray(Wo, np.float32)).astype(np.float32)

    pp, mm = np.meshgrid(np.arange(P), np.arange(S // 64), indexing="ij")
    base_k = (mm * 16 + (pp % 16)).astype(np.int16)     # j = m*16 + lane

    in_maps = []
    for c in range(NCORES):
        b, h = divmod(c, 2)
        q_rows = tokens[b, h * SQ:(h + 1) * SQ]
        in_maps.append({
            "tokTq": np.ascontiguousarray(q_rows.T.astype(bf)),  # [D, SQ]
            "resid": np.ascontiguousarray((q_rows + bo_eff).astype(bf)),
            "wq": wq_b, "wk": wk_b, "wv": wv_b, "wo": wo_b,
            "bq": bq, "bk": bk, "bv": bv,
            "gidxk": np.ascontiguousarray(base_k + np.int16((1 - h) * (SQ // 2))),
        })
    return in_maps


def gather_out(results):
    out = np.empty((B, S, D), np.float32)
    for c in range(NCORES):
        b, h = divmod(c, 2)
        out[b, h * SQ:(h + 1) * SQ] = results[c]["out"]
    return out


def kernel(tokens, Wq, bq, Wk, bk, Wv, bv, Wo, bo):
    from concourse.bass_utils import run_bass_kernel_spmd

    in_maps = make_in_maps(tokens, Wq, bq, Wk, bk, Wv, bv, Wo, bo)
    nc = _get_program()
    res = run_bass_kernel_spmd(nc, in_maps, core_ids=list(range(NCORES)),
                               trace=False)
    return gather_out(res.results)


# revision 29
# speedup vs baseline: 1.1425x; 1.0144x over previous
"""Trainium2 Bass kernel for a single-head attention block (B=4, S=2048, D=1024).

reference:
    x = gelu(tokens); q,k,v = x@W{q,k,v} + b; scores = q@k^T/sqrt(D)
    out = softmax(scores)@v @ Wo + bo + tokens

Sharding: 8 cores = 4 batches x 2 query-halves. Core c=2b+h handles batch b and
query rows [h*1024, (h+1)*1024). Each core computes q/k/v projections for its
own rows only; K^T and V halves are exchanged pairwise via two AllGathers. The
fp32 residual path dominates the output magnitude, so the whole matmul pipeline
runs in fp8-e4m3 with DoubleRow perf mode (K=256 per matmul).

KEY LAYOUT TRICK (v4): softmax is permutation-invariant over the key axis as
long as k and v use the SAME order, so each core keeps its OWN key/value rows
in tiles [0, SQ) of kT/v and the PEER's rows in [SQ, 2*SQ). The projection
evictions write straight into the own half (no copy), and the peer half is
pulled from the AllGather output with a dma_gather whose int16 row indices are
HOST-PROVIDED per-core data (peer slot = 1-h) — the program stays SPMD-uniform
while the own-half scores run with no dependency on the collective at all.

Scales: weights are pre-scaled x32 on the host (sigma~1 in fp8), so stored
q,k,v are 32x true scale. scores_psum = 32768*scores_true -> exp uses
scale=2^-15, bias=-5ln2, giving expT = exp(scores)/32 in fp8. Softmax
denominators via a ones-stationary matmul; rS_row = 1/Sigma exp is broadcast
across partitions via a DRAM round-trip (hidden behind the other chunk's
scores). The mixed psum is normalized on the psum->fp8 DVE convert (v is
centered host-side so the fp8 mixUT quantizes the small AC part). The out-proj
psum is 32*(mixed@Wo)*32, folded by 1/1024 on the fused
(psum*c + residual) DVE op; bo and the centering correction are pre-added into
the residual on the host.

Schedule: PSUM evictions alternate ACT/DVE; PE order is
  warmup | kTo -> AG1 | vo -> AG2 | qT | sc0-own sc0-peer S0 | sc1-own
  sc1-peer S1 | mix0 out0 | mix1 out1
so the AllGather wire+gather latency hides behind qT+own-half scores, and each
chunk's softmax reciprocal round-trip hides behind the other chunk's work.
"""

import math

import numpy as np
import ml_dtypes

B, S, D = 4, 2048, 1024
NCORES = 8
SQ = S // 2          # query rows per core
P = 128
DT = 8               # d / 128
KP = DT // 2         # K-pair count for DoubleRow (K=256 each)
ST = S // P          # 16 seq tiles
SQT = SQ // P        # 8
N512 = 512
WARMUP_MMS = 34
WSCALE = 32.0        # host-side weight/bias scale
EXP_BIAS = -5.0 * math.log(2.0)   # expT = exp(scores)/32
EXP_SCALE = 1.0 / 32768.0         # scores_psum = 32768 * scores_true
OUT_DESCALE = 1.0 / 1024.0
GELU_MEAN = 0.3989422804014327    # E[gelu(z)], z ~ N(0,1)

_COMPILED = {}


def _build_program():
    from contextlib import ExitStack

    import concourse.bass as bass
    import concourse.tile as tile
    from concourse import bacc, mybir

    f32 = mybir.dt.float32
    bf16 = mybir.dt.bfloat16
    f8 = mybir.dt.float8e4
    i16 = mybir.dt.int16
    AF = mybir.ActivationFunctionType
    ALU = mybir.AluOpType
    DR = mybir.MatmulPerfMode.DoubleRow

    nc = bacc.Bacc("TRN2", target_bir_lowering=False, debug=False,
                   num_devices=NCORES)

    tokTq = nc.dram_tensor("tokTq", [D, SQ], bf16, kind="ExternalInput")
    resid = nc.dram_tensor("resid", [SQ, D], bf16, kind="ExternalInput")
    wq = nc.dram_tensor("wq", [D, D], f8, kind="ExternalInput")
    wk = nc.dram_tensor("wk", [D, D], f8, kind="ExternalInput")
    wv = nc.dram_tensor("wv", [D, D], f8, kind="ExternalInput")
    wo = nc.dram_tensor("wo", [D, D], f8, kind="ExternalInput")
    bq_d = nc.dram_tensor("bq", [D], f32, kind="ExternalInput")   # x32
    bk_d = nc.dram_tensor("bk", [D], f32, kind="ExternalInput")   # x32
    bv_d = nc.dram_tensor("bv", [D], f32, kind="ExternalInput")   # x32
    gidxk_d = nc.dram_tensor("gidxk", [P, S // 64], i16, kind="ExternalInput")
    out_d = nc.dram_tensor("out", [SQ, D], f32, kind="ExternalOutput")

    ts = bass.ts
    groups = [[2 * i, 2 * i + 1] for i in range(NCORES // 2)]

    with tile.TileContext(nc) as tc, ExitStack() as ctx:
        pers = ctx.enter_context(tc.tile_pool(name="pers", bufs=1))
        kT = pers.tile([P, DT, SQ], f8, tag="kT")     # own keys
        kTp = pers.tile([P, DT, SQ], f8, tag="kTp")   # peer keys
        qT = pers.tile([P, DT, SQ], f8, tag="qT")
        v = pers.tile([P, SQT, D], f8, tag="v")       # own values
        vp = pers.tile([P, SQT, D], f8, tag="vp")     # peer values
        ones = pers.tile([P, 2, 16], f8, tag="ones")
        bqk = pers.tile([P, 2, DT], f32, tag="bqk")  # [:,0,:]=32bq [:,1,:]=32bk
        ebias = pers.tile([P, 1], f32, tag="ebias")
        wscr = pers.tile([P, N512], bf16, tag="wscr")
        wsink = pers.tile([P, P], f32, tag="wsink")
        wo_sb = pers.tile([P, DT, D], f8, tag="wo")
        gidxk = pers.tile([P, S // 64], i16, tag="gidxk")

        dram = ctx.enter_context(tc.tile_pool(name="dram", bufs=1, space="DRAM"))
        kb_in_a = dram.tile([D // 2, SQ], f8, tag="kb_in_a")
        kb_in_b = dram.tile([D // 2, SQ], f8, tag="kb_in_b")
        kb_out_a = dram.tile([2, D // 2, SQ], f8, tag="kb_out_a")
        kb_out_b = dram.tile([2, D // 2, SQ], f8, tag="kb_out_b")
        vb_in_a = dram.tile([SQ // 2, D], f8, tag="vb_in_a")
        vb_in_b = dram.tile([SQ // 2, D], f8, tag="vb_in_b")
        vb_out_a = dram.tile([2, SQ // 2, D], f8, tag="vb_out_a")
        vb_out_b = dram.tile([2, SQ // 2, D], f8, tag="vb_out_b")

        psum = ctx.enter_context(tc.tile_pool(name="psum", bufs=6, space="PSUM"))
        psum_s = ctx.enter_context(tc.tile_pool(name="psum_s", bufs=2, space="PSUM"))

        # --- PE warm-up: dense trivial matmuls so HAM hits K=8/8 and PE is
        # busy while the gelu+DMA head runs.
        nc.vector.memset(wscr, 0.0)
        wps = psum.tile([P, N512], f32, tag="mm")
        for i in range(WARMUP_MMS):
            nc.tensor.matmul(wps, wscr[:, :P], wscr, start=(i == 0),
                             stop=(i == WARMUP_MMS - 1))
        nc.vector.tensor_copy(wsink, wps[:, :P])

        nc.vector.memset(ones, 1.0)
        nc.vector.memset(ebias, EXP_BIAS)
        nc.scalar.dma_start(bqk[:, 0, :], bq_d.ap().rearrange("(t p) -> p t", p=P))
        nc.scalar.dma_start(bqk[:, 1, :], bk_d.ap().rearrange("(t p) -> p t", p=P))

        # ---------------- phase 1: gelu + projections + kT/v exchange -------
        with ExitStack() as ph1:
            p1 = ph1.enter_context(tc.tile_pool(name="p1", bufs=1))
            xTq = p1.tile([P, DT, SQ], f8, tag="xTq")
            wk_sb = p1.tile([P, DT, D], f8, tag="wk")
            wq_sb = p1.tile([P, DT, D], f8, tag="wq")
            wv_sb = p1.tile([P, DT, D], f8, tag="wv")
            bv_sb = p1.tile([P, D], f32, tag="bv")
            stag = ph1.enter_context(tc.tile_pool(name="stag", bufs=4))

            # Head is HBM-bound: load ONLY what the gelu needs now (tokens +
            # Wk); Wv/Wq/Wo triggers are interleaved into the staging loops
            # below so their 3MB doesn't steal HBM bandwidth from the tokens.
            nc.gpsimd.dma_start(wk_sb,
                                wk.ap().rearrange("(t p) e -> p t e", p=P))
            nc.gpsimd.dma_start(
                bv_sb, bass.AP(tensor=bv_d, offset=0, ap=[[0, P], [1, D]]))
            nc.gpsimd.dma_start(gidxk, gidxk_d.ap())
            # tokens in 4 pair-tiles; gelu per pair so each ACT op unlocks a
            # full DoubleRow K-pair for the projection matmuls
            for g in range(DT // 2):
                stq = stag.tile([P, 2, SQ], bf16, tag="tok", name=f"stq{g}")
                nc.sync.dma_start(
                    stq, tokTq.ap()[2 * g * P:(2 * g + 2) * P, :].rearrange(
                        "(t p) s -> p t s", p=P))
                nc.scalar.activation(xTq[:, 2 * g:2 * g + 2, :], stq, AF.Gelu)

            # kTo: lhsT = Wk-slice, rhs = xTq -> write own half of kT
            # directly; stream each te row-block to DRAM as its converts
            # land. The exchange is split into two half-AllGathers so the
            # first fires as soon as te 0-3 are staged.
            kb_in_av = kb_in_a[:].rearrange("(t p) s -> p t s", p=P)
            kb_in_bv = kb_in_b[:].rearrange("(t p) s -> p t s", p=P)
            for te in range(DT):
                for c in range(SQ // N512):
                    ps = psum.tile([P, N512], f32, tag="mm")
                    for u in range(KP):
                        nc.tensor.matmul(ps, wk_sb[:, 2 * u:2 * u + 2, ts(te, P)],
                                         xTq[:, 2 * u:2 * u + 2, ts(c, N512)],
                                         start=(u == 0), stop=(u == KP - 1),
                                         perf_mode=DR)
                    if c == 0:
                        nc.scalar.activation(kT[:, te, ts(c, N512)], ps,
                                             AF.Identity, bias=bqk[:, 1, te:te + 1])
                    else:
                        nc.vector.tensor_scalar_add(kT[:, te, ts(c, N512)], ps,
                                                    bqk[:, 1, te:te + 1])
                kb_v = kb_in_av if te < 4 else kb_in_bv
                nc.sync.dma_start(kb_v[:, te % 4, :], kT[:, te, :])
                if te == 0:
                    nc.sync.dma_start(
                        wv_sb, wv.ap().rearrange("(t p) e -> p t e", p=P))
                elif te == 3:
                    nc.sync.dma_start(
                        wq_sb, wq.ap().rearrange("(t p) e -> p t e", p=P))
                elif te == DT - 1:
                    pass
                if te == 3:
                    nc.gpsimd.collective_compute(
                        "AllGather", mybir.AluOpType.bypass,
                        replica_groups=groups,
                        ins=[kb_in_a[:].opt()], outs=[kb_out_a[:].opt()])
            nc.gpsimd.collective_compute(
                "AllGather", mybir.AluOpType.bypass, replica_groups=groups,
                ins=[kb_in_b[:].opt()], outs=[kb_out_b[:].opt()])

            # vo : lhsT = xTq-slice, rhs = Wv -> own half of v; exchange
            # split in two half-AllGathers like the keys
            vb_in_av = vb_in_a[:].rearrange("(t p) d -> p t d", p=P)
            vb_in_bv = vb_in_b[:].rearrange("(t p) d -> p t d", p=P)
            for tsq in range(SQT):
                for dc in range(D // N512):
                    ps = psum.tile([P, N512], f32, tag="mm")
                    for u in range(KP):
                        nc.tensor.matmul(ps, xTq[:, 2 * u:2 * u + 2, ts(tsq, P)],
                                         wv_sb[:, 2 * u:2 * u + 2, ts(dc, N512)],
                                         start=(u == 0), stop=(u == KP - 1),
                                         perf_mode=DR)
                    nc.vector.tensor_add(v[:, tsq, ts(dc, N512)], ps,
                                         bv_sb[:, ts(dc, N512)])
                vb_v = vb_in_av if tsq < 4 else vb_in_bv
                nc.sync.dma_start(vb_v[:, tsq % 4, :], v[:, tsq, :])
                if tsq == 1:
                    nc.sync.dma_start(
                        wo_sb, wo.ap().rearrange("(t p) e -> p t e", p=P))
                if tsq == 3:
                    nc.gpsimd.collective_compute(
                        "AllGather", mybir.AluOpType.bypass,
                        replica_groups=groups,
                        ins=[vb_in_a[:].opt()], outs=[vb_out_a[:].opt()])
            nc.gpsimd.collective_compute(
                "AllGather", mybir.AluOpType.bypass, replica_groups=groups,
                ins=[vb_in_b[:].opt()], outs=[vb_out_b[:].opt()])

            # peer halves: gather the peer's rows of the AllGather outputs
            # straight into the peer tiles — idx data is per-core
            nc.gpsimd.dma_gather(kTp[:, 0:4, :],
                                 kb_out_a[:].rearrange("r d s -> (r d) s"),
                                 gidxk[:, :], S // 4, S // 4, SQ)
            nc.gpsimd.dma_gather(kTp[:, 4:8, :],
                                 kb_out_b[:].rearrange("r d s -> (r d) s"),
                                 gidxk[:, :], S // 4, S // 4, SQ)
            nc.gpsimd.dma_gather(vp[:, 0:4, :],
                                 vb_out_a[:].rearrange("r s d -> (r s) d"),
                                 gidxk[:, :], S // 4, S // 4, D)
            # second v-half in two 256KB gathers: the tk-major mix peer
            # loop needs vp tiles 4-5 first, so let them land earlier
            nc.gpsimd.dma_gather(vp[:, 4:6, :],
                                 vb_out_b[:].rearrange("r s d -> (r s) d"),
                                 gidxk[:, 0:16], S // 8, S // 8, D)
            nc.gpsimd.dma_gather(vp[:, 6:8, :],
                                 vb_out_b[:].rearrange("r s d -> (r s) d"),
                                 gidxk[:, 16:32], S // 8, S // 8, D)

            # qT : lhsT = Wq-slice, rhs = xTq
            for te in range(DT):
                for c in range(SQ // N512):
                    ps = psum.tile([P, N512], f32, tag="mm")
                    for u in range(KP):
                        nc.tensor.matmul(ps, wq_sb[:, 2 * u:2 * u + 2, ts(te, P)],
                                         xTq[:, 2 * u:2 * u + 2, ts(c, N512)],
                                         start=(u == 0), stop=(u == KP - 1),
                                         perf_mode=DR)
                    if c == 0:
                        nc.scalar.activation(qT[:, te, ts(c, N512)], ps,
                                             AF.Identity, bias=bqk[:, 0, te:te + 1])
                    else:
                        nc.vector.tensor_scalar_add(qT[:, te, ts(c, N512)], ps,
                                                    bqk[:, 0, te:te + 1])

        # ---------------- phase 2: attention + out-proj ----------------
        with ExitStack() as ph2:
            epool = ph2.enter_context(tc.tile_pool(name="ep", bufs=2))
            work = ph2.enter_context(tc.tile_pool(name="wk2", bufs=2))
            opool = ph2.enter_context(tc.tile_pool(name="op2", bufs=2))
            rspool = ph2.enter_context(tc.tile_pool(name="rs2", bufs=2))
            rpool = ph2.enter_context(tc.tile_pool(name="rp", bufs=8))
            dpool = ph2.enter_context(
                tc.tile_pool(name="dram2", bufs=2, space="DRAM"))

            # scores in own-half / peer-half blocks; each chunk's softmax
            # denominator + reciprocal round-trip hides behind later blocks
            expTs, rSbs = [], []
            for c in range(SQ // N512):          # sq chunks of 512
                expT = epool.tile([P, ST, N512], f8, tag="expT",
                                  name=f"expT{c}")
                expTs.append(expT)

            def sc_block(c, tk_lo, tk_hi):
                expT = expTs[c]
                for tk in range(tk_lo, tk_hi):
                    ksrc = kT if tk < SQT else kTp
                    ps = psum.tile([P, N512], f32, tag="mm")
                    for u in range(KP):
                        nc.tensor.matmul(ps,
                                         ksrc[:, 2 * u:2 * u + 2,
                                              ts(tk % SQT, P)],
                                         qT[:, 2 * u:2 * u + 2, ts(c, N512)],
                                         start=(u == 0), stop=(u == KP - 1),
                                         perf_mode=DR)
                    nc.scalar.activation(expT[:, tk, :], ps, AF.Exp,
                                         scale=EXP_SCALE, bias=ebias)

            def s_block(c):
                expT = expTs[c]
                psS = psum_s.tile([1, N512], f32, tag="S")
                for tk in range(ST // 2):
                    nc.tensor.matmul(psS, ones[:, :, :1],
                                     expT[:, 2 * tk:2 * tk + 2, :],
                                     start=(tk == 0), stop=(tk == ST // 2 - 1),
                                     perf_mode=DR)
                rS_row = rspool.tile([1, N512], f32, tag="rS_row",
                                     name=f"rS{c}")
                nc.vector.reciprocal(rS_row, psS)   # = 32 / Sigma exp
                # broadcast 1/S across partitions via DRAM (stride-0 DMA)
                rs_dram = dpool.tile([N512], f32, tag="rs_dram")
                nc.sync.dma_start(
                    rs_dram[:].rearrange("(o s) -> o s", o=1), rS_row)
                rSb = rspool.tile([P, N512], f32, tag="rSb", name=f"rSb{c}")
                nc.scalar.dma_start(rSb, rs_dram[:].partition_broadcast(P))
                rSbs.append(rSb)

            sc_block(0, 0, SQT)        # own keys: no collective dependency
            sc_block(1, 0, SQT)        # more own-key work to hide the wire
            sc_block(0, SQT, ST)       # peer keys: needs AG1 + gathers
            s_block(0)
            sc_block(1, SQT, ST)
            s_block(1)

            # residual prefetch AFTER the exchange window so its HBM reads
            # don't fight the AllGather wire + gathers (bf16: half traffic)
            res_sbs = []
            for sl8 in range(SQT):
                res_sb = rpool.tile([P, D], bf16, tag="res", name=f"res{sl8}")
                nc.sync.dma_start(res_sb, resid.ap()[sl8 * P:(sl8 + 1) * P, :])
                res_sbs.append(res_sb)

            for c in range(SQ // N512):
                expT, rSb = expTs[c], rSbs[c]
                # mixedUT[d, sq] = (v^T-stationary @ expT) / S  (normalized on
                # the psum->fp8 convert; unnormalized would overflow e4m3).
                # For chunk 0 the peer values may still be in flight, so six
                # dsl groups run their own-half accumulations first (banks
                # held open) to cover the tail of the v-exchange with work.
                mixUT = work.tile([P, DT, N512], f8, tag="mixUT",
                                  name=f"mixUT{c}")
                nheld = 6 if c == 0 else 0
                held = []
                for dsl in range(nheld):
                    ps = psum.tile([P, N512], f32, tag="mm")
                    for tk in range(SQT // 2):
                        nc.tensor.matmul(ps, v[:, 2 * tk:2 * tk + 2, ts(dsl, P)],
                                         expT[:, 2 * tk:2 * tk + 2, :],
                                         start=(tk == 0), stop=False,
                                         perf_mode=DR)
                    held.append(ps)
                # peer half tk-major: all held groups consume the early
                # vp tiles (first half-gather) before any needs the late ones
                for tk in range(SQT // 2, ST // 2):
                    for dsl in range(nheld):
                        nc.tensor.matmul(held[dsl],
                                         vp[:, (2 * tk) % SQT:
                                             (2 * tk) % SQT + 2, ts(dsl, P)],
                                         expT[:, 2 * tk:2 * tk + 2, :],
                                         start=False, stop=(tk == ST // 2 - 1),
                                         perf_mode=DR)
                for dsl in range(nheld):
                    nc.vector.tensor_mul(mixUT[:, dsl, :], held[dsl], rSb)
                for dsl in range(nheld, DT):
                    ps = psum.tile([P, N512], f32, tag="mm")
                    for tk in range(ST // 2):
                        vsrc = v if tk < SQT // 2 else vp
                        nc.tensor.matmul(ps,
                                         vsrc[:, (2 * tk) % SQT:
                                              (2 * tk) % SQT + 2, ts(dsl, P)],
                                         expT[:, 2 * tk:2 * tk + 2, :],
                                         start=(tk == 0), stop=(tk == ST // 2 - 1),
                                         perf_mode=DR)
                    nc.vector.tensor_mul(mixUT[:, dsl, :], ps, rSb)

                for sl in range(4):
                    row = (c * 4 + sl) * P
                    res_sb = res_sbs[c * 4 + sl]
                    out_sb = opool.tile([P, D], f32, tag="osb")
                    osc = opool.tile([P, N512], f32, tag="osc")
                    for ec in range(D // N512):
                        ps = psum.tile([P, N512], f32, tag="mm")
                        for u in range(KP):
                            nc.tensor.matmul(
                                ps, mixUT[:, 2 * u:2 * u + 2, ts(sl, P)],
                                wo_sb[:, 2 * u:2 * u + 2, ts(ec, N512)],
                                start=(u == 0), stop=(u == KP - 1),
                                perf_mode=DR)
                        # out = psum / 1024 + (residual + bo); alternate the
                        # evict between DVE (fused) and ACT+GpSimd
                        if ec == 0:
                            nc.vector.scalar_tensor_tensor(
                                out_sb[:, ts(ec, N512)], ps, OUT_DESCALE,
                                res_sb[:, ts(ec, N512)], ALU.mult, ALU.add)
                        else:
                            nc.scalar.activation(osc, ps, AF.Identity,
                                                 scale=OUT_DESCALE)
                            nc.gpsimd.tensor_add(out_sb[:, ts(ec, N512)], osc,
                                                 res_sb[:, ts(ec, N512)])
                    nc.sync.dma_start(out_d.ap()[row:row + P, :], out_sb)

    nc.compile()
    return nc


def _get_program():
    if "nc" not in _COMPILED:
        _COMPILED["nc"] = _build_program()
    return _COMPILED["nc"]


def make_in_maps(tokens, Wq, bq, Wk, bk, Wv, bv, Wo, bo):
    tokens = np.asarray(tokens, dtype=np.float32)
    bf = ml_dtypes.bfloat16
    f8 = ml_dtypes.float8_e4m3
    wq_b = np.ascontiguousarray((np.asarray(Wq, np.float32) * WSCALE).astype(f8))
    wk_b = np.ascontiguousarray((np.asarray(Wk, np.float32) * WSCALE).astype(f8))
    wv_b = np.ascontiguousarray((np.asarray(Wv, np.float32) * WSCALE).astype(f8))
    wo_b = np.ascontiguousarray((np.asarray(Wo, np.float32) * WSCALE).astype(f8))
    bq = np.asarray(bq, np.float32) * WSCALE
    bk = np.asarray(bk, np.float32) * WSCALE
    # center v by c ~ E_k[v] so the fp8 mixUT quantizes the small AC part;
    # softmax weights sum to 1, so out = (mixed-c)@Wo + (c@Wo + bo) + resid.
    wv32 = np.asarray(Wv, np.float32)
    cvec = GELU_MEAN * wv32.sum(axis=0) + np.asarray(bv, np.float32)
    bv = (np.asarray(bv, np.float32) - cvec) * WSCALE
    bo_eff = (np.asarray(bo, np.float32)
              + cvec @ np.asarray(Wo, np.float32)).astype(np.float32)

    pp, mm = np.meshgrid(np.arange(P), np.arange(S // 64), indexing="ij")
    base_k = (mm * 16 + (pp % 16)).astype(np.int16)     # j = m*16 + lane

    in_maps = []
    for c in range(NCORES):
        b, h = divmod(c, 2)
        q_rows = tokens[b, h * SQ:(h + 1) * SQ]
        in_maps.append({
            "tokTq": np.ascontiguousarray(q_rows.T.astype(bf)),  # [D, SQ]
            "resid": np.ascontiguousarray((q_rows + bo_eff).astype(bf)),
            "wq": wq_b, "wk": wk_b, "wv": wv_b, "wo": wo_b,
            "bq": bq, "bk": bk, "bv": bv,
            "gidxk": np.ascontiguousarray(base_k + np.int16((1 - h) * (SQ // 2))),
        })
    return in_maps


def gather_out(results):
    out = np.empty((B, S, D), np.float32)
    for c in range(NCORES):
        b, h = divmod(c, 2)
        out[b, h * SQ:(h + 1) * SQ] = results[c]["out"]
    return out


def kernel(tokens, Wq, bq, Wk, bk, Wv, bv, Wo, bo):
    from concourse.bass_utils import run_bass_kernel_spmd

    in_maps = make_in_maps(tokens, Wq, bq, Wk, bk, Wv, bv, Wo, bo)
    nc = _get_program()
    res = run_bass_kernel_spmd(nc, in_maps, core_ids=list(range(NCORES)),
                               trace=False)
    return gather_out(res.results)
